# revision 5
# baseline (speedup 1.0000x reference)
"""KNN max-pooling kernel for Trainium2 (8 NeuronCores, SPMD).

out[m, :] = max_{s<16} feat[idx[m, s], :]   feat: [100000, 64] f32, idx: [100000, 16] i64

Strategy: shard the 100000 query rows across 8 cores (12500 each), replicate
the feature table in DRAM. Each core fetches neighbors with the gpsimd SWDGE
dma_gather. dma_gather indices are int16 (max 32767), so the table is viewed
as 25000 "quad" rows of 4x64 f32 (1KB): quad index = idx>>2 always fits, no
chunking and no invalid indices. The wanted row inside each quad is selected
with a host-precomputed {0, -3e38} additive mask, then a pairwise max tree on
the vector engine reduces quads and the 16 neighbors. Double-buffered tiles.
"""

import sys

if "/opt/trn_rl_repo" not in sys.path:
    sys.path.insert(0, "/opt/trn_rl_repo")

import numpy as np

import concourse.bacc as bacc
import concourse.tile as tile
from concourse import bass, mybir
from concourse.bass_utils import run_bass_kernel_spmd

# Problem shape (hardcoded per contract).
N_POINTS = 100000
N_QUERY = 100000
NSAMPLE = 16
C = 64

N_CORES = 8
M_LOC = N_QUERY // N_CORES   # 12500 queries per core
P = 128                      # queries per block (one per partition)
NB = 2                       # blocks per supertile
SUPER = P * NB               # queries per supertile
T = -(-M_LOC // SUPER)       # supertiles per core
M_PAD = T * SUPER

NQUAD = N_POINTS // 4        # 25000 quad rows (< 32768, int16-safe)
QC = 4 * C                   # 256 f32 per quad row
NSLOT = NB * NSAMPLE         # gather slots per partition per supertile
NIDX = NSLOT * P             # gather descriptors per supertile
NEG = -3.0e38

_CACHE = {}


def _ap(base, offset, dims):
    """Free-dim view of a tile: partition dim from base, custom (stride, n)."""
    a = base[:]
    return bass.AP(a.tensor, offset, [a.ap[0]] + list(dims))


def _build_program():
    nc = bacc.Bacc("TRN2", target_bir_lowering=False, debug=False)
    feat_t = nc.dram_tensor("feat", [NQUAD, QC], mybir.dt.float32,
                            kind="ExternalInput")
    idx_t = nc.dram_tensor("idx_dev", [T, P, NIDX // 16], mybir.dt.int16,
                           kind="ExternalInput")
    mask_t = nc.dram_tensor("mask_dev", [T, P, NSLOT, 4], mybir.dt.float32,
                            kind="ExternalInput")
    out_t = nc.dram_tensor("out_dev", [T, P, NB, C], mybir.dt.float32,
                           kind="ExternalOutput")

    with tile.TileContext(nc) as tc:
        with tc.tile_pool(name="big", bufs=2) as bigp, \
             tc.tile_pool(name="small", bufs=3) as smallp:
            for t in range(T):
                idx_tile = smallp.tile([P, NIDX // 16], mybir.dt.int16, tag="idx")
                nc.sync.dma_start(idx_tile[:], idx_t[t, :, :])
                mask_tile = smallp.tile([P, NSLOT, 4], mybir.dt.float32, tag="mask")
                nc.sync.dma_start(mask_tile[:], mask_t[t, :, :, :])

                # stage[p, slot, :] = quad row for (query block*128+p, neighbor)
                # slot = b*16 + s; gather idx j = slot*128 + p
                # SWDGE ring holds 128 entries of 16 descs -> <=2032 idx/call
                stage = bigp.tile([P, NSLOT, QC], mybir.dt.float32, tag="stage")
                GIDX = 1024
                for g in range(NIDX // GIDX):
                    nc.gpsimd.dma_gather(
                        out_ap=stage[:, g * (GIDX // P):(g + 1) * (GIDX // P), :],
                        in_ap=feat_t[:],
                        idxs_ap=idx_tile[:, g * (GIDX // 16):(g + 1) * (GIDX // 16)],
                        num_idxs=GIDX,
                        num_idxs_reg=GIDX,
                        elem_size=QC,
                        queue_num=0,
                    )

                # kill the 3 unwanted rows of each quad: stage += mask (bcast C)
                st4 = _ap(stage, 0, [(QC, NSLOT), (C, 4), (1, C)])
                mb4 = _ap(mask_tile, 0, [(4, NSLOT), (1, 4), (0, C)])
                nc.vector.tensor_tensor(out=st4, in0=st4, in1=mb4,
                                        op=mybir.AluOpType.add)

                # collapse quad: 4x64 -> 2x64 -> 1x64
                a01 = _ap(stage, 0, [(QC, NSLOT), (1, 2 * C)])
                a23 = _ap(stage, 2 * C, [(QC, NSLOT), (1, 2 * C)])
                nc.vector.tensor_tensor(out=a01, in0=a01, in1=a23,
                                        op=mybir.AluOpType.max)
                red = smallp.tile([P, NSLOT, C], mybir.dt.float32, tag="red")
                r0 = _ap(stage, 0, [(QC, NSLOT), (1, C)])
                r1 = _ap(stage, C, [(QC, NSLOT), (1, C)])
                nc.vector.tensor_tensor(out=red[:], in0=r0, in1=r1,
                                        op=mybir.AluOpType.max)

                # neighbor max tree within each block of 16 slots
                out_tile = smallp.tile([P, NB, C], mybir.dt.float32, tag="out")
                for b in range(NB):
                    s0 = b * NSAMPLE
                    w = NSAMPLE // 2
                    while w >= 2:
                        nc.vector.tensor_tensor(
                            out=red[:, s0:s0 + w, :],
                            in0=red[:, s0:s0 + w, :],
                            in1=red[:, s0 + w:s0 + 2 * w, :],
                            op=mybir.AluOpType.max,
                        )
                        w //= 2
                    nc.vector.tensor_tensor(
                        out=out_tile[:, b:b + 1, :],
                        in0=red[:, s0:s0 + 1, :],
                        in1=red[:, s0 + 1:s0 + 2, :],
                        op=mybir.AluOpType.max,
                    )
                nc.sync.dma_start(out_t[t, :, :, :], out_tile[:])

    nc.compile()
    return nc


def _prep_inputs(idx):
    """idx [100000,16] -> per-core (idx_dev int16 wrapped, mask_dev f32)."""
    idxq = (idx >> 2).astype(np.int16)
    rem = (idx & 3).astype(np.int64)
    idx_devs, mask_devs = [], []
    for k in range(N_CORES):
        q = np.zeros((M_PAD, NSAMPLE), np.int16)
        r = np.zeros((M_PAD, NSAMPLE), np.int64)
        q[:M_LOC] = idxq[k * M_LOC:(k + 1) * M_LOC]
        r[:M_LOC] = rem[k * M_LOC:(k + 1) * M_LOC]
        # flat gather index j = (b*16+s)*128 + p  ->  [T, NB, S, P]
        arr = q.reshape(T, NB, P, NSAMPLE).transpose(0, 1, 3, 2)
        flat = arr.reshape(T, NIDX)
        # idx j lives at (partition j%16, column j//16), replicated x8
        wrapped = flat.reshape(T, NIDX // 16, 16).transpose(0, 2, 1)
        idx_devs.append(np.ascontiguousarray(np.tile(wrapped, (1, 8, 1))))
        # mask[t, p, slot=(b,s), k] = 0 if k == rem else NEG
        rr = r.reshape(T, NB, P, NSAMPLE).transpose(0, 2, 1, 3).reshape(T, P, NSLOT)
        m = np.where(np.arange(4)[None, None, None, :] == rr[..., None],
                     np.float32(0.0), np.float32(NEG)).astype(np.float32)
        mask_devs.append(np.ascontiguousarray(m))
    return idx_devs, mask_devs


def _unshard_out(outs):
    parts = []
    for o in outs:
        full = o.reshape(T, P, NB, C).transpose(0, 2, 1, 3).reshape(M_PAD, C)
        parts.append(full[:M_LOC])
    return np.concatenate(parts, axis=0)


def run(feat, idx, trace=False):
    if "nc" not in _CACHE:
        _CACHE["nc"] = _build_program()
    nc = _CACHE["nc"]

    featq = np.ascontiguousarray(
        feat.astype(np.float32, copy=False).reshape(NQUAD, QC))
    idx_devs, mask_devs = _prep_inputs(idx)
    in_maps = [{"feat": featq, "idx_dev": idx_devs[k], "mask_dev": mask_devs[k]}
               for k in range(N_CORES)]

    res = run_bass_kernel_spmd(nc, in_maps, core_ids=list(range(N_CORES)),
                               trace=trace)
    out = _unshard_out([r["out_dev"] for r in res.results])
    return out, res.exec_time_ns


def kernel(feat, idx):
    out, _ = run(feat, idx, trace=False)
    return out


# revision 6
# speedup vs baseline: 1.7621x; 1.7621x over previous
"""KNN max-pooling kernel for Trainium2 (8 NeuronCores, SPMD).

out[m, :] = max_{s<16} feat[idx[m, s], :]   feat: [100000, 64] f32, idx: [100000, 16] i64

Strategy: shard the 100000 query rows across 8 cores (12500 each), replicate
the feature table in DRAM. Each core fetches neighbors with the gpsimd SWDGE
dma_gather. dma_gather indices are int16 (max 32767), so the table is viewed
as 25000 "quad" rows of 4x64 f32 (1KB): quad index = idx>>2 always fits, no
chunking and no invalid indices. The wanted row inside each quad is selected
with a host-precomputed {0, -3e38} additive mask, then a pairwise max tree on
the vector engine reduces quads and the 16 neighbors. Double-buffered tiles.
"""

import sys

if "/opt/trn_rl_repo" not in sys.path:
    sys.path.insert(0, "/opt/trn_rl_repo")

import numpy as np

import concourse.bacc as bacc
import concourse.tile as tile
from concourse import bass, mybir
from concourse.bass_utils import run_bass_kernel_spmd

# Problem shape (hardcoded per contract).
N_POINTS = 100000
N_QUERY = 100000
NSAMPLE = 16
C = 64

N_CORES = 8
M_LOC = N_QUERY // N_CORES   # 12500 queries per core
P = 128                      # queries per block (one per partition)
NB = 2                       # blocks per supertile
SUPER = P * NB               # queries per supertile
T = -(-M_LOC // SUPER)       # supertiles per core
M_PAD = T * SUPER

NQUAD = N_POINTS // 4        # 25000 quad rows (< 32768, int16-safe)
QC = 4 * C                   # 256 f32 per quad row
NSLOT = NB * NSAMPLE         # gather slots per partition per supertile
NIDX = NSLOT * P             # gather descriptors per supertile
NEG = -3.0e38

_CACHE = {}


def _ap(base, offset, dims):
    """Free-dim view of a tile: partition dim from base, custom (stride, n)."""
    a = base[:]
    return bass.AP(a.tensor, offset, [a.ap[0]] + list(dims))


def _build_program():
    nc = bacc.Bacc("TRN2", target_bir_lowering=False, debug=False,
                   num_swdge_queues=4)
    feat_t = nc.dram_tensor("feat", [NQUAD, QC], mybir.dt.float32,
                            kind="ExternalInput")
    idx_t = nc.dram_tensor("idx_dev", [T, P, NIDX // 16], mybir.dt.int16,
                           kind="ExternalInput")
    mask_t = nc.dram_tensor("mask_dev", [T, P, NSLOT, 4], mybir.dt.float32,
                            kind="ExternalInput")
    out_t = nc.dram_tensor("out_dev", [T, P, NB, C], mybir.dt.float32,
                           kind="ExternalOutput")

    with tile.TileContext(nc) as tc:
        with tc.tile_pool(name="big", bufs=2) as bigp, \
             tc.tile_pool(name="small", bufs=3) as smallp:
            for t in range(T):
                idx_tile = smallp.tile([P, NIDX // 16], mybir.dt.int16, tag="idx")
                nc.sync.dma_start(idx_tile[:], idx_t[t, :, :])
                mask_tile = smallp.tile([P, NSLOT, 4], mybir.dt.float32, tag="mask")
                nc.sync.dma_start(mask_tile[:], mask_t[t, :, :, :])

                # stage[p, slot, :] = quad row for (query block*128+p, neighbor)
                # slot = b*16 + s; gather idx j = slot*128 + p
                # SWDGE ring holds 128 entries of 16 descs -> <=2032 idx/call
                stage = bigp.tile([P, NSLOT, QC], mybir.dt.float32, tag="stage")
                GIDX = 1024
                for g in range(NIDX // GIDX):
                    nc.gpsimd.dma_gather(
                        out_ap=stage[:, g * (GIDX // P):(g + 1) * (GIDX // P), :],
                        in_ap=feat_t[:],
                        idxs_ap=idx_tile[:, g * (GIDX // 16):(g + 1) * (GIDX // 16)],
                        num_idxs=GIDX,
                        num_idxs_reg=GIDX,
                        elem_size=QC,
                        queue_num=(t * (NIDX // GIDX) + g) % 4,
                    )

                # kill the 3 unwanted rows of each quad: stage += mask (bcast C)
                st4 = _ap(stage, 0, [(QC, NSLOT), (C, 4), (1, C)])
                mb4 = _ap(mask_tile, 0, [(4, NSLOT), (1, 4), (0, C)])
                nc.vector.tensor_tensor(out=st4, in0=st4, in1=mb4,
                                        op=mybir.AluOpType.add)

                # collapse quad: 4x64 -> 2x64 -> 1x64
                a01 = _ap(stage, 0, [(QC, NSLOT), (1, 2 * C)])
                a23 = _ap(stage, 2 * C, [(QC, NSLOT), (1, 2 * C)])
                nc.vector.tensor_tensor(out=a01, in0=a01, in1=a23,
                                        op=mybir.AluOpType.max)
                red = smallp.tile([P, NSLOT, C], mybir.dt.float32, tag="red")
                r0 = _ap(stage, 0, [(QC, NSLOT), (1, C)])
                r1 = _ap(stage, C, [(QC, NSLOT), (1, C)])
                nc.vector.tensor_tensor(out=red[:], in0=r0, in1=r1,
                                        op=mybir.AluOpType.max)

                # neighbor max tree within each block of 16 slots
                out_tile = smallp.tile([P, NB, C], mybir.dt.float32, tag="out")
                for b in range(NB):
                    s0 = b * NSAMPLE
                    w = NSAMPLE // 2
                    while w >= 2:
                        nc.vector.tensor_tensor(
                            out=red[:, s0:s0 + w, :],
                            in0=red[:, s0:s0 + w, :],
                            in1=red[:, s0 + w:s0 + 2 * w, :],
                            op=mybir.AluOpType.max,
                        )
                        w //= 2
                    nc.vector.tensor_tensor(
                        out=out_tile[:, b:b + 1, :],
                        in0=red[:, s0:s0 + 1, :],
                        in1=red[:, s0 + 1:s0 + 2, :],
                        op=mybir.AluOpType.max,
                    )
                nc.sync.dma_start(out_t[t, :, :, :], out_tile[:])

    nc.compile()
    return nc


def _prep_inputs(idx):
    """idx [100000,16] -> per-core (idx_dev int16 wrapped, mask_dev f32)."""
    idxq = (idx >> 2).astype(np.int16)
    rem = (idx & 3).astype(np.int64)
    idx_devs, mask_devs = [], []
    for k in range(N_CORES):
        q = np.zeros((M_PAD, NSAMPLE), np.int16)
        r = np.zeros((M_PAD, NSAMPLE), np.int64)
        q[:M_LOC] = idxq[k * M_LOC:(k + 1) * M_LOC]
        r[:M_LOC] = rem[k * M_LOC:(k + 1) * M_LOC]
        # flat gather index j = (b*16+s)*128 + p  ->  [T, NB, S, P]
        arr = q.reshape(T, NB, P, NSAMPLE).transpose(0, 1, 3, 2)
        flat = arr.reshape(T, NIDX)
        # idx j lives at (partition j%16, column j//16), replicated x8
        wrapped = flat.reshape(T, NIDX // 16, 16).transpose(0, 2, 1)
        idx_devs.append(np.ascontiguousarray(np.tile(wrapped, (1, 8, 1))))
        # mask[t, p, slot=(b,s), k] = 0 if k == rem else NEG
        rr = r.reshape(T, NB, P, NSAMPLE).transpose(0, 2, 1, 3).reshape(T, P, NSLOT)
        m = np.where(np.arange(4)[None, None, None, :] == rr[..., None],
                     np.float32(0.0), np.float32(NEG)).astype(np.float32)
        mask_devs.append(np.ascontiguousarray(m))
    return idx_devs, mask_devs


def _unshard_out(outs):
    parts = []
    for o in outs:
        full = o.reshape(T, P, NB, C).transpose(0, 2, 1, 3).reshape(M_PAD, C)
        parts.append(full[:M_LOC])
    return np.concatenate(parts, axis=0)


def run(feat, idx, trace=False):
    if "nc" not in _CACHE:
        _CACHE["nc"] = _build_program()
    nc = _CACHE["nc"]

    featq = np.ascontiguousarray(
        feat.astype(np.float32, copy=False).reshape(NQUAD, QC))
    idx_devs, mask_devs = _prep_inputs(idx)
    in_maps = [{"feat": featq, "idx_dev": idx_devs[k], "mask_dev": mask_devs[k]}
               for k in range(N_CORES)]

    res = run_bass_kernel_spmd(nc, in_maps, core_ids=list(range(N_CORES)),
                               trace=trace)
    out = _unshard_out([r["out_dev"] for r in res.results])
    return out, res.exec_time_ns


def kernel(feat, idx):
    out, _ = run(feat, idx, trace=False)
    return out
